# revision 1
# baseline (speedup 1.0000x reference)
"""Fused linear + cross-entropy loss (BaseChunkLoss) on 8 trn2 NeuronCores.

Strategy (per the sharding hint: token/data parallel):
  - Tokens (N=8192) are sharded 8 ways: each core handles 1024 tokens x the
    full vocab (32000), so every core computes a complete logsumexp for its
    tokens and no cross-device reduction of partials is needed.
  - head_weight streams through each core (262 MB fp32 -> ~360 GB/s DMA,
    overlapped with compute); the 1024-token hidden slice stays resident in
    SBUF.
  - The final tiny reduction - log(s), nll = lse - tgt, weighted mean, and
    the 8-way scalar combine - happens on host, standing in for the
    wrapper's all_reduce of the scalar loss.

Device kernel layout: tokens on PSUM partitions, vocab on the free dim.
  stationary lhsT = hidden^T tile [128 d x 128 tok]
  moving rhs      = weight^T tile [128 d x 500 vocab]
  psum [128 tok x 500 vocab] fp32, accumulated over the D=2048 contraction.
Matmuls run in fp8e4m3 with perf_mode=DoubleRow (2 contraction rows per PE
cell, K=256 per instruction; weights pre-scaled by 64 on-chip for e4m3
range, descaled during the bias add). Per 1500-wide vocab group: DVE does
(psum/64 + bias) in place, DVE extracts the target logit via
(iota == label) * logit with a fused row-sum accumulator, and ACT computes
exp with a fused row-sum accumulator. Set USE_FP8 = False for a bf16
variant (~2.5e-6 loss error instead of ~5e-5, ~1.7x slower).

Host-side input prep is layout-only (transpose/slice/cast of index arrays);
all FLOPs over hidden/weights happen on device inside the measured kernel.
"""
import numpy as np
from contextlib import ExitStack

from concourse import bacc, mybir, tile
from concourse.bass_utils import run_bass_kernel_spmd

F32 = mybir.dt.float32
BF16 = mybir.dt.bfloat16
FP8 = mybir.dt.float8e4
Alu = mybir.AluOpType
Act = mybir.ActivationFunctionType

USE_FP8 = True

N_CORES = 8
N_TOK = 8192
D = 2048
V = 32000
P = 128
KT = D // P            # 16 k-tiles of 128
BANK = 500             # vocab columns per psum bank (<= 512 fp32)
BPG = 3                # banks per vocab group
T = N_TOK // N_CORES   # 1024 tokens per core
T_CONST = T
V_CONST = V
MB = T // P            # 8 token blocks per core

W_SCALE = 64.0         # fp8 weight pre-scale (e4m3 range)
WPAD = 1536            # fp8 W tile inner stride (multiple of 16 for DoubleRow)


def _vocab_groups():
    nbanks = V // BANK
    groups = []
    b = 0
    while b < nbanks:
        nb = min(BPG, nbanks - b)
        groups.append((b * BANK, nb * BANK, nb, b))
        b += nb
    return groups


def _declare_io(nc):
    # h and W arrive pre-transposed from host: h [D, T], W [D, V]
    return (
        nc.declare_dram_parameter("h", [D, T], F32, isOutput=False),
        nc.declare_dram_parameter("W", [D, V], F32, isOutput=False),
        nc.declare_dram_parameter("bias", [V], F32, isOutput=False),
        nc.declare_dram_parameter("iota", [V], F32, isOutput=False),
        nc.declare_dram_parameter("labs", [P, MB], F32, isOutput=False),
        nc.declare_dram_parameter("s_out", [P, MB], F32, isOutput=True),
        nc.declare_dram_parameter("t_out", [P, MB], F32, isOutput=True),
    )


def _postops(nc, pt, nb, nv, bb, ii, labs_t, m, col, junk, ejunk,
             s_cols, t_cols, descale):
    psl = pt[:, 0:nb, 0:BANK]
    bbv = bb[:, 0:nv].rearrange("p (b c) -> p b c", c=BANK)
    iiv = ii[:, 0:nv].rearrange("p (b c) -> p b c", c=BANK)
    if descale:
        nc.vector.scalar_tensor_tensor(
            psl, psl, 1.0 / W_SCALE, bbv, op0=Alu.mult, op1=Alu.add)
    else:
        nc.vector.tensor_tensor(psl, psl, bbv, op=Alu.add)
    jt = junk.tile([P, BPG, BANK], F32, tag="junk")
    nc.vector.scalar_tensor_tensor(
        jt[:, 0:nb, :], iiv, labs_t[:, m:m + 1], psl,
        op0=Alu.is_equal, op1=Alu.mult,
        accum_out=t_cols[:, col:col + 1],
    )
    et = ejunk.tile([P, BPG, BANK], F32, tag="ejunk")
    nc.scalar.activation(
        et[:, 0:nb, :], psl, Act.Exp, accum_out=s_cols[:, col:col + 1])


def _finish(nc, acc, s_cols, t_cols, ng, s_out, t_out):
    s_fin = acc.tile([P, MB], F32, tag="sfin")
    t_fin = acc.tile([P, MB], F32, tag="tfin")
    for m in range(MB):
        nc.vector.tensor_reduce(
            s_fin[:, m:m + 1], s_cols[:, m * ng:(m + 1) * ng],
            axis=mybir.AxisListType.X, op=Alu.add)
        nc.vector.tensor_reduce(
            t_fin[:, m:m + 1], t_cols[:, m * ng:(m + 1) * ng],
            axis=mybir.AxisListType.X, op=Alu.add)
    nc.sync.dma_start(s_out[:], s_fin[:])
    nc.sync.dma_start(t_out[:], t_fin[:])


def _build_bf16():
    groups = _vocab_groups()
    ng = len(groups)
    nc = bacc.Bacc("TRN2", target_bir_lowering=False, debug=False)
    h_d, W_d, bias_d, iota_d, labs_d, s_out, t_out = _declare_io(nc)
    W_r = W_d[:].rearrange("(ko ki) v -> ko ki v", ki=P)   # [KT, 128, V]
    h_r = h_d[:].rearrange("(ko ki) t -> ko ki t", ki=P)   # [KT, 128, T]

    with tile.TileContext(nc) as tc, ExitStack() as ctx:
        hpool = ctx.enter_context(tc.tile_pool(name="hT", bufs=1))
        hstage = ctx.enter_context(tc.tile_pool(name="hstage", bufs=2))
        wpool = ctx.enter_context(tc.tile_pool(name="w", bufs=3))
        wstage = ctx.enter_context(tc.tile_pool(name="wstage", bufs=2))
        bpool = ctx.enter_context(tc.tile_pool(name="bias", bufs=2))
        ipool = ctx.enter_context(tc.tile_pool(name="iota", bufs=2))
        pspool = ctx.enter_context(tc.tile_pool(name="ps", bufs=2, space="PSUM"))
        junk = ctx.enter_context(tc.tile_pool(name="junk", bufs=2))
        ejunk = ctx.enter_context(tc.tile_pool(name="ejunk", bufs=2))
        acc = ctx.enter_context(tc.tile_pool(name="acc", bufs=1))

        labs_t = acc.tile([P, MB], F32, tag="labs")
        nc.sync.dma_start(labs_t[:], labs_d[:])
        s_cols = acc.tile([P, MB * ng], F32, tag="scols")
        t_cols = acc.tile([P, MB * ng], F32, tag="tcols")

        hT = hpool.tile([P, KT, T], BF16, tag="hT")
        for k in range(KT):
            st = hstage.tile([P, T], F32, tag="hstage")
            nc.sync.dma_start(st[:], h_r[k])
            nc.vector.tensor_copy(hT[:, k, :], st[:])

        for voff, nv, nb, col0 in groups:
            wv = wpool.tile([P, KT, BPG * BANK], BF16, tag="w")
            for k in range(KT):
                ws = wstage.tile([P, BPG * BANK], F32, tag="wstage")
                nc.sync.dma_start(ws[:, :nv], W_r[k, :, voff:voff + nv])
                nc.scalar.copy(wv[:, k, :nv], ws[:, :nv])
            bb = bpool.tile([P, BPG * BANK], F32, tag="bias")
            nc.scalar.dma_start(
                bb[:, :nv], bias_d[voff:voff + nv].partition_broadcast(P))
            ii = ipool.tile([P, BPG * BANK], F32, tag="iota")
            nc.scalar.dma_start(
                ii[:, :nv], iota_d[voff:voff + nv].partition_broadcast(P))

            for m in range(MB):
                pt = pspool.tile([P, BPG, 512], F32, tag="ps")
                for k in range(KT):
                    lhsT = hT[:, k, m * P:(m + 1) * P]
                    for bk in range(nb):
                        nc.tensor.matmul(
                            pt[:, bk, 0:BANK], lhsT,
                            wv[:, k, bk * BANK:(bk + 1) * BANK],
                            start=(k == 0), stop=(k == KT - 1),
                        )
                col = m * ng + (col0 // BPG)
                _postops(nc, pt, nb, nv, bb, ii, labs_t, m, col, junk, ejunk,
                         s_cols, t_cols, descale=False)

        _finish(nc, acc, s_cols, t_cols, ng, s_out, t_out)

    nc.compile()
    return nc


def _build_fp8():
    T, V = T_CONST, V_CONST
    """fp8 DoubleRow v5: 4 banks/group; tgt via exact f32 rowdot of gathered
    weight rows (host gathers W[labels]; device does the dot); drain chain is
    one DVE op + one ACT op per psum slot."""
    BPG4 = 4
    GV = BPG4 * BANK            # 2000 vocab per group
    WPAD4 = 2048
    assert V % GV == 0
    MB = T // P
    ng = V // GV
    KP2 = KT // 2

    nc = bacc.Bacc("TRN2", target_bir_lowering=False, debug=False)
    h_d = nc.declare_dram_parameter("h", [D, T], F32, isOutput=False)
    W_d = nc.declare_dram_parameter("W", [D, V], F32, isOutput=False)
    bias_d = nc.declare_dram_parameter("bias", [V], F32, isOutput=False)
    hn_d = nc.declare_dram_parameter("hn", [T, D], F32, isOutput=False)
    wg_d = nc.declare_dram_parameter("wg", [T, D], F32, isOutput=False)
    s_out = nc.declare_dram_parameter("s_out", [P, MB], F32, isOutput=True)
    t_out = nc.declare_dram_parameter("t_out", [P, MB], F32, isOutput=True)

    W_r2 = W_d[:].rearrange("(kp j ki) v -> kp ki j v", ki=P, j=2)
    h_r2 = h_d[:].rearrange("(kp j ki) t -> kp ki j t", ki=P, j=2)

    with tile.TileContext(nc) as tc, ExitStack() as ctx:
        hpool = ctx.enter_context(tc.tile_pool(name="hT", bufs=1))
        hstage = ctx.enter_context(tc.tile_pool(name="hstage", bufs=2))
        wpool = ctx.enter_context(tc.tile_pool(name="w", bufs=2))
        wstage = ctx.enter_context(tc.tile_pool(name="wstage", bufs=2))
        bpool = ctx.enter_context(tc.tile_pool(name="bias", bufs=2))
        gpool = ctx.enter_context(tc.tile_pool(name="gath", bufs=2))
        pspool = ctx.enter_context(tc.tile_pool(name="ps", bufs=2, space="PSUM"))
        ejunk = ctx.enter_context(tc.tile_pool(name="ejunk", bufs=1))
        djunk = ctx.enter_context(tc.tile_pool(name="djunk", bufs=1))
        acc = ctx.enter_context(tc.tile_pool(name="acc", bufs=1))

        s_cols = acc.tile([P, MB * ng], F32, tag="scols")
        t_fin = acc.tile([P, MB], F32, tag="tfin")

        # exact-f32 target logit: per m-block rowdot of hn and gathered rows
        for m in range(MB):
            hg = gpool.tile([P, D], F32, tag="hg")
            nc.scalar.dma_start(hg[:], hn_d[m * P:(m + 1) * P, :])
            wgt = gpool.tile([P, D], F32, tag="wgt")
            nc.scalar.dma_start(wgt[:], wg_d[m * P:(m + 1) * P, :])
            dj = djunk.tile([P, D], F32, tag="djunk")
            nc.vector.tensor_mul(dj[:], hg[:], wgt[:])
            nc.vector.tensor_reduce(
                t_fin[:, m:m + 1], dj[:], axis=mybir.AxisListType.X, op=Alu.add)

        hT = hpool.tile([P, KP2, 2, T], FP8, tag="hT")
        for kp in range(KP2):
            st = hstage.tile([P, 2, T], F32, tag="hstage")
            nc.sync.dma_start(st[:], h_r2[kp])
            nc.vector.tensor_copy(hT[:, kp, :, :], st[:])

        for g in range(ng):
            voff = g * GV
            wv = wpool.tile([P, KP2, 2, WPAD4], FP8, tag="w")
            for kp in range(KP2):
                ws = wstage.tile([P, 2, GV], F32, tag="wstage")
                nc.sync.dma_start(ws[:], W_r2[kp][:, :, voff:voff + GV])
                if kp % 2 == 0:
                    nc.scalar.mul(wv[:, kp, :, 0:GV], ws[:], W_SCALE)
                else:
                    nc.vector.tensor_scalar_mul(wv[:, kp, :, 0:GV], ws[:], W_SCALE)
            bb = bpool.tile([P, GV], F32, tag="bias")
            nc.scalar.dma_start(bb[:], bias_d[voff:voff + GV].partition_broadcast(P))

            for m in range(MB):
                pt = pspool.tile([P, BPG4, 512], F32, tag="ps")
                for kp in range(KP2):
                    lhsT = hT[:, kp, :, m * P:(m + 1) * P]
                    for bk in range(BPG4):
                        nc.tensor.matmul(
                            pt[:, bk, 0:BANK], lhsT,
                            wv[:, kp, :, bk * BANK:(bk + 1) * BANK],
                            start=(kp == 0), stop=(kp == KP2 - 1),
                            perf_mode=mybir.MatmulPerfMode.DoubleRow,
                        )
                col = m * ng + g
                psl = pt[:, 0:BPG4, 0:BANK]
                bbv = bb[:].rearrange("p (b c) -> p b c", c=BANK)
                nc.vector.scalar_tensor_tensor(
                    psl, psl, 1.0 / W_SCALE, bbv, op0=Alu.mult, op1=Alu.add)
                et = ejunk.tile([P, BPG4, BANK], F32, tag="ejunk")
                nc.scalar.activation(
                    et[:], psl, Act.Exp, accum_out=s_cols[:, col:col + 1])

        s_fin = acc.tile([P, MB], F32, tag="sfin")
        for m in range(MB):
            nc.vector.tensor_reduce(
                s_fin[:, m:m + 1], s_cols[:, m * ng:(m + 1) * ng],
                axis=mybir.AxisListType.X, op=Alu.add)
        nc.sync.dma_start(s_out[:], s_fin[:])
        nc.sync.dma_start(t_out[:], t_fin[:])

    nc.compile()
    return nc


_NC_CACHE = {}


def _get_program():
    key = "fp8" if USE_FP8 else "bf16"
    if key not in _NC_CACHE:
        _NC_CACHE[key] = _build_fp8() if USE_FP8 else _build_bf16()
    return _NC_CACHE[key]


def kernel(hidden_states, head_weight, head_bias, loss_weight, labels,
           chunk_size=None, **_unused):
    hidden = np.asarray(hidden_states, dtype=np.float32)
    W = np.asarray(head_weight, dtype=np.float32)
    bias = np.asarray(head_bias, dtype=np.float32)
    lw = np.asarray(loss_weight, dtype=np.float32)
    labels = np.asarray(labels)

    assert hidden.shape == (N_TOK, D) and W.shape == (V, D)

    nc = _get_program()
    Wt = np.ascontiguousarray(W.T)                 # [D, V]
    ht = np.ascontiguousarray(hidden.T)            # [D, N]
    in_maps = []
    if USE_FP8:
        Wg = W[labels.astype(np.int64)]            # gathered rows [N, D]
        for c in range(N_CORES):
            sl = slice(c * T, (c + 1) * T)
            in_maps.append(dict(
                h=np.ascontiguousarray(ht[:, sl]), W=Wt, bias=bias,
                hn=np.ascontiguousarray(hidden[sl]),
                wg=np.ascontiguousarray(Wg[sl])))
    else:
        iota = np.arange(V, dtype=np.float32)
        for c in range(N_CORES):
            sl = slice(c * T, (c + 1) * T)
            labs = labels[sl].reshape(MB, P).T.astype(np.float32).copy()
            in_maps.append(dict(h=np.ascontiguousarray(ht[:, sl]), W=Wt,
                                bias=bias, iota=iota, labs=labs))
    res = run_bass_kernel_spmd(nc, in_maps, list(range(N_CORES)))

    # unshard + host-side scalar combine (the "all_reduce" of the hint)
    s = np.concatenate([r["s_out"].T.reshape(-1) for r in res.results])
    tgt = np.concatenate([r["t_out"].T.reshape(-1) for r in res.results])
    if USE_FP8:
        # device produced the exact f32 dot h.W[label]; add the bias here
        tgt = tgt + bias[labels.astype(np.int64)]
    lse = np.log(s.astype(np.float64))
    nll = lse - tgt.astype(np.float64)
    w64 = lw.astype(np.float64)
    loss = (w64 * nll).sum() / max(w64.sum(), 1.0)
    return np.float32(loss)



# revision 2
# speedup vs baseline: 2.1436x; 2.1436x over previous
"""Fused linear + cross-entropy loss (BaseChunkLoss) on 8 trn2 NeuronCores.

Strategy (vocab/tensor parallel, per the sharding hint's second option):
  - head_weight is sharded over vocab: each core owns a 4000-column slice
    of W (and bias) and computes, for ALL 8192 tokens, the partial
    sum-of-exp over its vocab slice.  The cross-device "logsumexp" is the
    host-side combine: s[tok] = sum_c s_c[tok], lse = log(s).
  - Per-core HBM traffic is ~3x lower than token-sharding (hidden 67MB +
    W slice 33MB vs. full W 262MB), which moves the kernel from DMA-bound
    to PE-bound at the fp8 DoubleRow matmul floor (~426us/core).
  - f32 -> fp8 conversion of both hidden and W happens inside the casting
    software-DGE DMA (gpsimd queue), so no staging buffers or on-chip
    conversion passes are needed.  W is cast unscaled; e4m3 subnormal
    rounding gives per-element absolute error comparable to the pre-scaled
    variant (measured end-to-end loss rel-err ~3e-5).
  - The target logit is computed exactly in f32 as a row-dot of the
    host-gathered rows W[labels] with hidden (token-sharded: each core
    does its own 1024 tokens), fused into one tensor_tensor_reduce per
    128-token block on DVE.  Host adds bias[labels] during the combine.

Device layout: tokens on PSUM partitions, vocab on the free dim.
  lhsT = hidden^T fp8 tile [128 d-pairs x 2 x 128 tok]  (stationary)
  rhs  = W^T      fp8 tile [128 d-pairs x 2 x 500 vocab] (moving)
  psum [128 tok x 4 banks x 500 vocab] f32, accumulated over D=2048 in 8
  DoubleRow steps (K=256 per matmul).
Drain per psum tile: DVE writes junk_bf16 = psum + bias (this frees the
psum bank pair in ~2.5us, under the 3.3us PE fill time, so PE never
stalls), then ACT computes exp(junk) with a fused row-sum accumulator
into s_cols, off the critical path.

Host-side prep is layout-only (transpose/slice/gather); all FLOPs over
hidden/weights happen on device inside the measured kernel.  The final
scalar combine (log, weighted mean, 8-way add) is the wrapper's
all_reduce stand-in.
"""
import numpy as np
from contextlib import ExitStack

from concourse import bacc, mybir, tile
from concourse.bass_utils import run_bass_kernel_spmd

F32 = mybir.dt.float32
BF16 = mybir.dt.bfloat16
FP8 = mybir.dt.float8e4
Alu = mybir.AluOpType
Act = mybir.ActivationFunctionType

N_CORES = 8
N_TOK = 8192
D = 2048
V = 32000
P = 128

VC = V // N_CORES       # 4000 vocab columns per core
GV = 2000               # vocab columns per drain group
NG = VC // GV           # 2 groups
BANKS = 4               # psum banks per group
BANK = GV // BANKS      # 500 vocab columns per bank
KP = D // 256           # 8 DoubleRow contraction steps (K=256 each)
TB = 1024               # tokens per streamed hidden block
NB = N_TOK // TB        # 8 blocks
MBB = TB // P           # 8 psum m-blocks per hidden block
MG = N_TOK // P         # 64 global m-blocks (s output columns)
TC = N_TOK // N_CORES   # 1024 tokens per core for the exact row-dot
MR = TC // P            # 8 row-dot m-blocks


def _build():
    nc = bacc.Bacc("TRN2", target_bir_lowering=False, debug=False)
    # h and W arrive pre-transposed from host: h [D, N], W [D, VC]
    h_d = nc.declare_dram_parameter("h", [D, N_TOK], F32, isOutput=False)
    W_d = nc.declare_dram_parameter("W", [D, VC], F32, isOutput=False)
    bias_d = nc.declare_dram_parameter("bias", [VC], F32, isOutput=False)
    hn_d = nc.declare_dram_parameter("hn", [TC, D], F32, isOutput=False)
    wg_d = nc.declare_dram_parameter("wg", [TC, D], F32, isOutput=False)
    s_out = nc.declare_dram_parameter("s_out", [P, MG], F32, isOutput=True)
    t_out = nc.declare_dram_parameter("t_out", [P, MR], F32, isOutput=True)

    W_r = W_d[:].rearrange("(kp j ki) v -> kp ki j v", ki=P, j=2)  # [8,128,2,VC]
    h_r = h_d[:].rearrange("(kp j ki) t -> kp ki j t", ki=P, j=2)  # [8,128,2,N]

    with tile.TileContext(nc) as tc, ExitStack() as ctx:
        wpool = ctx.enter_context(tc.tile_pool(name="w", bufs=1))
        hpool = ctx.enter_context(tc.tile_pool(name="hT", bufs=2))
        bpool = ctx.enter_context(tc.tile_pool(name="bias", bufs=1))
        gpool = ctx.enter_context(tc.tile_pool(name="gath", bufs=2))
        dpool = ctx.enter_context(tc.tile_pool(name="dj", bufs=1))
        jpool = ctx.enter_context(tc.tile_pool(name="jt", bufs=3))
        epool = ctx.enter_context(tc.tile_pool(name="et", bufs=2))
        pspool = ctx.enter_context(tc.tile_pool(name="ps", bufs=2, space="PSUM"))
        acc = ctx.enter_context(tc.tile_pool(name="acc", bufs=1))

        s_cols = acc.tile([P, MG * NG], F32, tag="scols")
        s_fin = acc.tile([P, MG], F32, tag="sfin")
        t_fin = acc.tile([P, MR], F32, tag="tfin")

        wv = wpool.tile([P, KP, 2, VC], FP8, tag="w")
        bb = bpool.tile([P, VC], F32, tag="bias")

        def load_h(b):
            ht = hpool.tile([P, KP, 2, TB], FP8, tag="hT")
            for kp in range(KP):
                nc.gpsimd.dma_start(
                    ht[:, kp, :, :], h_r[kp][:, :, b * TB:(b + 1) * TB])
            return ht

        # Issue order on the gpsimd queue == DMA order: W group 0, hidden
        # block 0 (these gate the first matmul), then bias, W group 1,
        # hidden block 1.
        for kp in range(KP):
            nc.gpsimd.dma_start(wv[:, kp, :, 0:GV], W_r[kp][:, :, 0:GV])
        ht_cur = load_h(0)
        nc.gpsimd.dma_start(bb[:], bias_d[:].partition_broadcast(P))
        for kp in range(KP):
            nc.gpsimd.dma_start(wv[:, kp, :, GV:VC], W_r[kp][:, :, GV:VC])

        for b in range(NB):
            ht = ht_cur
            ht_next = load_h(b + 1) if b + 1 < NB else None
            # exact-f32 target logit for this core's token m-block b
            hg = gpool.tile([P, D], F32, tag="hg")
            nc.gpsimd.dma_start(hg[:], hn_d[b * P:(b + 1) * P, :])
            wgt = gpool.tile([P, D], F32, tag="wg")
            nc.gpsimd.dma_start(wgt[:], wg_d[b * P:(b + 1) * P, :])
            dj = dpool.tile([P, D], F32, tag="dj")
            nc.vector.tensor_tensor_reduce(
                dj[:], hg[:], wgt[:], 1.0, 0.0,
                op0=Alu.mult, op1=Alu.add, accum_out=t_fin[:, b:b + 1])

            for g in range(NG):
                bbv = bb[:, g * GV:(g + 1) * GV].rearrange(
                    "p (k c) -> p k c", c=BANK)
                for m in range(MBB):
                    pt = pspool.tile([P, BANKS, 512], F32, tag="ps")
                    for kp in range(KP):
                        lhsT = ht[:, kp, :, m * P:(m + 1) * P]
                        for bk in range(BANKS):
                            nc.tensor.matmul(
                                pt[:, bk, 0:BANK], lhsT,
                                wv[:, kp, :,
                                   g * GV + bk * BANK:g * GV + (bk + 1) * BANK],
                                start=(kp == 0), stop=(kp == KP - 1),
                                perf_mode=mybir.MatmulPerfMode.DoubleRow)
                    jt = jpool.tile([P, BANKS, BANK], BF16, tag="jt")
                    nc.vector.tensor_tensor(
                        jt[:], pt[:, 0:BANKS, 0:BANK], bbv, op=Alu.add)
                    et = epool.tile([P, BANKS, BANK], BF16, tag="et")
                    col = (b * MBB + m) * NG + g
                    nc.scalar.activation(
                        et[:], jt[:], Act.Exp,
                        accum_out=s_cols[:, col:col + 1])
            ht_cur = ht_next

        sv = s_cols[:].rearrange("p (m g) -> p m g", g=NG)
        nc.vector.tensor_tensor(s_fin[:], sv[:, :, 0], sv[:, :, 1], op=Alu.add)
        nc.sync.dma_start(s_out[:], s_fin[:])
        nc.sync.dma_start(t_out[:], t_fin[:])

    nc.compile()
    return nc


_NC_CACHE = {}


def _get_program():
    if "v2" not in _NC_CACHE:
        _NC_CACHE["v2"] = _build()
    return _NC_CACHE["v2"]


def kernel(hidden_states, head_weight, head_bias, loss_weight, labels,
           chunk_size=None, **_unused):
    hidden = np.asarray(hidden_states, dtype=np.float32)
    W = np.asarray(head_weight, dtype=np.float32)
    bias = np.asarray(head_bias, dtype=np.float32)
    lw = np.asarray(loss_weight, dtype=np.float32)
    labels = np.asarray(labels).astype(np.int64)

    assert hidden.shape == (N_TOK, D) and W.shape == (V, D)

    nc = _get_program()
    ht = np.ascontiguousarray(hidden.T)            # [D, N]
    Wt = np.ascontiguousarray(W.T)                 # [D, V]
    Wg = W[labels]                                 # gathered rows [N, D]
    in_maps = []
    for c in range(N_CORES):
        vsl = slice(c * VC, (c + 1) * VC)
        tsl = slice(c * TC, (c + 1) * TC)
        in_maps.append(dict(
            h=ht,
            W=np.ascontiguousarray(Wt[:, vsl]),
            bias=np.ascontiguousarray(bias[vsl]),
            hn=np.ascontiguousarray(hidden[tsl]),
            wg=np.ascontiguousarray(Wg[tsl])))
    res = run_bass_kernel_spmd(nc, in_maps, list(range(N_CORES)))

    # unshard + host-side scalar combine (the "all_reduce" of the hint):
    # sum the per-core partial exp-sums over vocab shards, then the
    # weighted-mean reduction over tokens.
    s = np.zeros((P, MG), dtype=np.float64)
    for r in res.results:
        s += r["s_out"].astype(np.float64)
    s = s.T.reshape(-1)                            # token-ordered [N]
    tgt = np.concatenate([r["t_out"].T.reshape(-1) for r in res.results])
    tgt = tgt.astype(np.float64) + bias[labels].astype(np.float64)
    lse = np.log(s)
    nll = lse - tgt
    w64 = lw.astype(np.float64)
    loss = (w64 * nll).sum() / max(w64.sum(), 1.0)
    return np.float32(loss)


# revision 14
# speedup vs baseline: 2.1686x; 1.0117x over previous
"""Fused linear + cross-entropy loss (BaseChunkLoss) on 8 trn2 NeuronCores.

Strategy (vocab/tensor parallel, per the sharding hint's second option):
  - head_weight is sharded over vocab: each core owns a 4000-column slice
    of W (and bias) and computes, for ALL 8192 tokens, the partial
    sum-of-exp over its vocab slice.  The cross-device "logsumexp" is the
    host-side combine: s[tok] = sum_c s_c[tok], lse = log(s).
  - Per-core HBM traffic is ~3x lower than token-sharding (hidden 67MB +
    W slice 33MB vs. full W 262MB), which moves the kernel from DMA-bound
    to PE-bound at the fp8 DoubleRow matmul floor (~426us/core).
  - f32 -> fp8 conversion of both hidden and W happens inside the casting
    software-DGE DMA (gpsimd queue), so no staging buffers or on-chip
    conversion passes are needed.  W is cast unscaled; e4m3 subnormal
    rounding gives per-element absolute error comparable to the pre-scaled
    variant (measured end-to-end loss rel-err ~3e-5).
  - The target logit is computed exactly in f32 as a row-dot of the
    host-gathered rows W[labels] with hidden (token-sharded: each core
    does its own 1024 tokens), fused into one tensor_tensor_reduce per
    128-token block on DVE.  Host adds bias[labels] during the combine.

Device layout: tokens on PSUM partitions, vocab on the free dim.
  lhsT = hidden^T fp8 tile [128 d-pairs x 2 x 128 tok]  (stationary)
  rhs  = W^T      fp8 tile [128 d-pairs x 2 x 500 vocab] (moving)
  psum [128 tok x 4 banks x 500 vocab] f32, accumulated over D=2048 in 8
  DoubleRow steps (K=256 per matmul).
Drain per psum tile: DVE writes junk_bf16 = psum + bias (this frees the
psum bank pair in ~2.5us, under the 3.3us PE fill time, so PE never
stalls), then ACT computes exp(junk) with a fused row-sum accumulator
into s_cols, off the critical path.

Host-side prep is layout-only (transpose/slice/gather); all FLOPs over
hidden/weights happen on device inside the measured kernel.  The final
scalar combine (log, weighted mean, 8-way add) is the wrapper's
all_reduce stand-in.
"""
import numpy as np
from contextlib import ExitStack

from concourse import bacc, mybir, tile
from concourse.bass_utils import run_bass_kernel_spmd

F32 = mybir.dt.float32
BF16 = mybir.dt.bfloat16
FP8 = mybir.dt.float8e4
Alu = mybir.AluOpType
Act = mybir.ActivationFunctionType

N_CORES = 8
N_TOK = 8192
D = 2048
V = 32000
P = 128

VC = V // N_CORES       # 4000 vocab columns per core
GV = 2000               # vocab columns per drain group
NG = VC // GV           # 2 groups
BANKS = 4               # psum banks per group
BANK = GV // BANKS      # 500 vocab columns per bank
KP = D // 256           # 8 DoubleRow contraction steps (K=256 each)
TB = 1024               # tokens per streamed hidden block
NB = N_TOK // TB        # 8 blocks
MBB = TB // P           # 8 psum m-blocks per hidden block
MG = N_TOK // P         # 64 global m-blocks (s output columns)
TC = N_TOK // N_CORES   # 1024 tokens per core for the exact row-dot
MR = TC // P            # 8 row-dot m-blocks

# startup gpsimd DMA order: ("w", (lo, hi)) | ("h", (lo, hi)) | ("b", group)
STARTUP_ORDER = (
    ("w", (0, GV)), ("h", (0, TB)), ("b", 0), ("b", 1), ("w", (GV, VC)),
)


def _build():
    nc = bacc.Bacc("TRN2", target_bir_lowering=False, debug=False)
    # h and W arrive pre-transposed from host: h [D, N], W [D, VC]
    h_d = nc.declare_dram_parameter("h", [D, N_TOK], F32, isOutput=False)
    W_d = nc.declare_dram_parameter("W", [D, VC], F32, isOutput=False)
    bias_d = nc.declare_dram_parameter("bias", [VC], F32, isOutput=False)
    hn_d = nc.declare_dram_parameter("hn", [TC, D], F32, isOutput=False)
    wg_d = nc.declare_dram_parameter("wg", [TC, D], F32, isOutput=False)
    s_out = nc.declare_dram_parameter("s_out", [P, MG], F32, isOutput=True)
    t_out = nc.declare_dram_parameter("t_out", [P, MR], F32, isOutput=True)

    W_r = W_d[:].rearrange("(kp j ki) v -> kp ki j v", ki=P, j=2)  # [8,128,2,VC]
    h_r = h_d[:].rearrange("(kp j ki) t -> kp ki j t", ki=P, j=2)  # [8,128,2,N]

    with tile.TileContext(nc) as tc, ExitStack() as ctx:
        wpool = ctx.enter_context(tc.tile_pool(name="w", bufs=1))
        hpool = ctx.enter_context(tc.tile_pool(name="hT", bufs=2))
        bpool = ctx.enter_context(tc.tile_pool(name="bias", bufs=1))
        gpool = ctx.enter_context(tc.tile_pool(name="gath", bufs=2))
        dpool = ctx.enter_context(tc.tile_pool(name="dj", bufs=1))
        jpool = ctx.enter_context(tc.tile_pool(name="jt", bufs=3))
        epool = ctx.enter_context(tc.tile_pool(name="et", bufs=2))
        pspool = ctx.enter_context(tc.tile_pool(name="ps", bufs=2, space="PSUM"))
        acc = ctx.enter_context(tc.tile_pool(name="acc", bufs=1))

        s_cols = acc.tile([P, MG * NG], F32, tag="scols")
        s_fin = acc.tile([P, MG], F32, tag="sfin")
        t_fin = acc.tile([P, MR], F32, tag="tfin")
        tpart = acc.tile([P, MR * 4], F32, tag="tpart")
        DC = D // 4             # row-dot chunk width

        wv = wpool.tile([P, KP, 2, VC], FP8, tag="w")
        bb = bpool.tile([P, VC], F32, tag="bias")

        def load_h(b):
            ht = hpool.tile([P, KP, 2, TB], FP8, tag="hT")
            for kp in range(KP):
                nc.gpsimd.dma_start(
                    ht[:, kp, :, :], h_r[kp][:, :, b * TB:(b + 1) * TB])
            return ht

        # Issue order on the gpsimd queue == DMA order.  The first psum
        # tile needs W group 0 + the first 128 tokens of hidden + bias
        # group 0; everything else overlaps with compute.
        ht_cur = hpool.tile([P, KP, 2, TB], FP8, tag="hT")

        def _dma_w(lo, hi):
            for kp in range(KP):
                nc.gpsimd.dma_start(
                    wv[:, kp, :, lo:hi], W_r[kp][:, :, lo:hi])

        def _dma_h0(lo, hi):
            for kp in range(KP):
                nc.gpsimd.dma_start(
                    ht_cur[:, kp, :, lo:hi], h_r[kp][:, :, lo:hi])

        def _dma_bias(g):
            nc.gpsimd.dma_start(
                bb[:, g * GV:(g + 1) * GV],
                bias_d[g * GV:(g + 1) * GV].partition_broadcast(P))

        # Startup DMA order (gpsimd queue == transfer order).  Chosen by
        # simulator sweep; the first psum tile needs W group 0 + the first
        # 128 tokens of hidden + bias group 0.
        for step in STARTUP_ORDER:
            kind, a = step
            if kind == "w":
                _dma_w(*a)
            elif kind == "h":
                _dma_h0(*a)
            else:
                _dma_bias(a)

        # Exact-f32 target-logit row-dot, chopped into D/4-wide chunks that
        # slot into the per-drain DVE slack (PE fills a psum tile in 3.33us,
        # the drain takes 2.2us; each 0.7us chunk fits the gap).  The
        # multiply chunks for block b run during b's g1 drains; the reduce
        # chunks run during block b+1's g0 drains (half-block lag so the
        # hg/wg DMAs always arrive in time).
        dj_prev = None
        for b in range(NB):
            ht = ht_cur
            hg = gpool.tile([P, D], F32, tag="hg")
            nc.gpsimd.dma_start(hg[:], hn_d[b * P:(b + 1) * P, :])
            wgt = gpool.tile([P, D], F32, tag="wg")
            nc.gpsimd.dma_start(wgt[:], wg_d[b * P:(b + 1) * P, :])
            ht_next = load_h(b + 1) if b + 1 < NB else None
            dj = dpool.tile([P, D], F32, tag="dj")

            for g in range(NG):
                bbv = bb[:, g * GV:(g + 1) * GV].rearrange(
                    "p (k c) -> p k c", c=BANK)
                for m in range(MBB):
                    pt = pspool.tile([P, BANKS, 512], F32, tag="ps")
                    for kp in range(KP):
                        lhsT = ht[:, kp, :, m * P:(m + 1) * P]
                        for bk in range(BANKS):
                            nc.tensor.matmul(
                                pt[:, bk, 0:BANK], lhsT,
                                wv[:, kp, :,
                                   g * GV + bk * BANK:g * GV + (bk + 1) * BANK],
                                start=(kp == 0), stop=(kp == KP - 1),
                                perf_mode=mybir.MatmulPerfMode.DoubleRow)
                    jt = jpool.tile([P, BANKS, BANK], BF16, tag="jt")
                    nc.vector.tensor_tensor(
                        jt[:], pt[:, 0:BANKS, 0:BANK], bbv, op=Alu.add)
                    et = epool.tile([P, BANKS, BANK], BF16, tag="et")
                    col = (b * MBB + m) * NG + g
                    nc.scalar.activation(
                        et[:], jt[:], Act.Exp,
                        accum_out=s_cols[:, col:col + 1])
                    # row-dot chunks in the drain slack.  Normally: muls of
                    # block b in b's g1 slots, reduces in b+1's g0 slots.
                    # The last block pulls both into its own slots so the
                    # tail has no row-dot work left.
                    last = b == NB - 1
                    if g == 0 and m < 4 and dj_prev is not None:
                        c = slice(m * DC, (m + 1) * DC)
                        nc.vector.tensor_reduce(
                            tpart[:, (b - 1) * 4 + m:(b - 1) * 4 + m + 1],
                            dj_prev[:, c], axis=mybir.AxisListType.X,
                            op=Alu.add)
                    if (g == 0 and 4 <= m if last else g == 1 and m < 4):
                        mm = m - 4 if last else m
                        c = slice(mm * DC, (mm + 1) * DC)
                        nc.vector.tensor_mul(dj[:, c], hg[:, c], wgt[:, c])
                    if last and g == 1 and m < 4:
                        c = slice(m * DC, (m + 1) * DC)
                        nc.vector.tensor_reduce(
                            tpart[:, b * 4 + m:b * 4 + m + 1],
                            dj[:, c], axis=mybir.AxisListType.X, op=Alu.add)
            dj_prev = dj
            ht_cur = ht_next

        tv = tpart[:].rearrange("p (m c) -> p m c", c=4)
        nc.vector.tensor_reduce(
            t_fin[:], tv, axis=mybir.AxisListType.X, op=Alu.add)
        sv = s_cols[:].rearrange("p (m g) -> p m g", g=NG)
        nc.vector.tensor_tensor(s_fin[:], sv[:, :, 0], sv[:, :, 1], op=Alu.add)
        nc.sync.dma_start(t_out[:], t_fin[:])
        nc.sync.dma_start(s_out[:], s_fin[:])

    nc.compile()
    return nc


_NC_CACHE = {}


def _get_program():
    if "v2" not in _NC_CACHE:
        _NC_CACHE["v2"] = _build()
    return _NC_CACHE["v2"]


def kernel(hidden_states, head_weight, head_bias, loss_weight, labels,
           chunk_size=None, **_unused):
    hidden = np.asarray(hidden_states, dtype=np.float32)
    W = np.asarray(head_weight, dtype=np.float32)
    bias = np.asarray(head_bias, dtype=np.float32)
    lw = np.asarray(loss_weight, dtype=np.float32)
    labels = np.asarray(labels).astype(np.int64)

    assert hidden.shape == (N_TOK, D) and W.shape == (V, D)

    nc = _get_program()
    ht = np.ascontiguousarray(hidden.T)            # [D, N]
    Wt = np.ascontiguousarray(W.T)                 # [D, V]
    Wg = W[labels]                                 # gathered rows [N, D]
    in_maps = []
    for c in range(N_CORES):
        vsl = slice(c * VC, (c + 1) * VC)
        tsl = slice(c * TC, (c + 1) * TC)
        in_maps.append(dict(
            h=ht,
            W=np.ascontiguousarray(Wt[:, vsl]),
            bias=np.ascontiguousarray(bias[vsl]),
            hn=np.ascontiguousarray(hidden[tsl]),
            wg=np.ascontiguousarray(Wg[tsl])))
    res = run_bass_kernel_spmd(nc, in_maps, list(range(N_CORES)))

    # unshard + host-side scalar combine (the "all_reduce" of the hint):
    # sum the per-core partial exp-sums over vocab shards, then the
    # weighted-mean reduction over tokens.
    s = np.zeros((P, MG), dtype=np.float64)
    for r in res.results:
        s += r["s_out"].astype(np.float64)
    s = s.T.reshape(-1)                            # token-ordered [N]
    tgt = np.concatenate([r["t_out"].T.reshape(-1) for r in res.results])
    tgt = tgt.astype(np.float64) + bias[labels].astype(np.float64)
    lse = np.log(s)
    nll = lse - tgt
    w64 = lw.astype(np.float64)
    loss = (w64 * nll).sum() / max(w64.sum(), 1.0)
    return np.float32(loss)


# revision 22
# speedup vs baseline: 2.1730x; 1.0020x over previous
"""Fused linear + cross-entropy loss (BaseChunkLoss) on 8 trn2 NeuronCores.

Strategy (vocab/tensor parallel, per the sharding hint's second option):
  - head_weight is sharded over vocab: each core owns a 4000-column slice
    of W (and bias) and computes, for ALL 8192 tokens, the partial
    sum-of-exp over its vocab slice.  The cross-device "logsumexp" is the
    host-side combine: s[tok] = sum_c s_c[tok], lse = log(s).
  - Per-core HBM traffic is ~3x lower than token-sharding (hidden 67MB +
    W slice 33MB vs. full W 262MB), which moves the kernel from DMA-bound
    to PE-bound at the fp8 DoubleRow matmul floor (~426us/core).
  - f32 -> fp8 conversion of both hidden and W happens inside the casting
    software-DGE DMA (gpsimd queue), so no staging buffers or on-chip
    conversion passes are needed.  W is cast unscaled; e4m3 subnormal
    rounding gives per-element absolute error comparable to the pre-scaled
    variant (measured end-to-end loss rel-err ~3e-5).
  - The target logit is computed exactly in f32 as a row-dot of the
    host-gathered rows W[labels] with hidden (token-sharded: each core
    does its own 1024 tokens), fused into one tensor_tensor_reduce per
    128-token block on DVE.  Host adds bias[labels] during the combine.

Device layout: tokens on PSUM partitions, vocab on the free dim.
  lhsT = hidden^T fp8 tile [128 d-pairs x 2 x 128 tok]  (stationary)
  rhs  = W^T      fp8 tile [128 d-pairs x 2 x 500 vocab] (moving)
  psum [128 tok x 4 banks x 500 vocab] f32, accumulated over D=2048 in 8
  DoubleRow steps (K=256 per matmul).
Drain per psum tile: DVE writes junk_bf16 = psum + bias (this frees the
psum bank pair in ~2.5us, under the 3.3us PE fill time, so PE never
stalls), then ACT computes exp(junk) with a fused row-sum accumulator
into s_cols, off the critical path.

Host-side prep is layout-only (transpose/slice/gather); all FLOPs over
hidden/weights happen on device inside the measured kernel.  The final
scalar combine (log, weighted mean, 8-way add) is the wrapper's
all_reduce stand-in.
"""
import numpy as np
from contextlib import ExitStack

from concourse import bacc, mybir, tile
from concourse.bass_utils import run_bass_kernel_spmd

F32 = mybir.dt.float32
BF16 = mybir.dt.bfloat16
FP8 = mybir.dt.float8e4
Alu = mybir.AluOpType
Act = mybir.ActivationFunctionType

N_CORES = 8
N_TOK = 8192
D = 2048
V = 32000
P = 128

VC = V // N_CORES       # 4000 vocab columns per core
GV = 2000               # vocab columns per drain group
NG = VC // GV           # 2 groups
BANKS = 4               # psum banks per group
BANK = GV // BANKS      # 500 vocab columns per bank
KP = D // 256           # 8 DoubleRow contraction steps (K=256 each)
TB = 1024               # tokens per streamed hidden block
NB = N_TOK // TB        # 8 blocks
MBB = TB // P           # 8 psum m-blocks per hidden block
MG = N_TOK // P         # 64 global m-blocks (s output columns)
TC = N_TOK // N_CORES   # 1024 tokens per core for the exact row-dot
MR = TC // P            # 8 row-dot m-blocks

# startup gpsimd DMA order: ("w", (lo, hi)) | ("h", (lo, hi)) | ("b", group)
STARTUP_ORDER = (
    ("w", (0, GV // 2)), ("h", (0, TB)), ("b", 0), ("w", (GV // 2, GV)),
    ("b", 1), ("w", (GV, VC)),
)


def _build():
    nc = bacc.Bacc("TRN2", target_bir_lowering=False, debug=False)
    # h and W arrive pre-transposed from host: h [D, N], W [D, VC]
    h_d = nc.declare_dram_parameter("h", [D, N_TOK], F32, isOutput=False)
    W_d = nc.declare_dram_parameter("W", [D, VC], F32, isOutput=False)
    bias_d = nc.declare_dram_parameter("bias", [VC], F32, isOutput=False)
    hn_d = nc.declare_dram_parameter("hn", [TC, D], F32, isOutput=False)
    wg_d = nc.declare_dram_parameter("wg", [TC, D], F32, isOutput=False)
    s_out = nc.declare_dram_parameter("s_out", [P, MG], F32, isOutput=True)
    t_out = nc.declare_dram_parameter("t_out", [P, MR], F32, isOutput=True)

    W_r = W_d[:].rearrange("(kp j ki) v -> kp ki j v", ki=P, j=2)  # [8,128,2,VC]
    h_r = h_d[:].rearrange("(kp j ki) t -> kp ki j t", ki=P, j=2)  # [8,128,2,N]

    with tile.TileContext(nc) as tc, ExitStack() as ctx:
        wpool = ctx.enter_context(tc.tile_pool(name="w", bufs=1))
        hpool = ctx.enter_context(tc.tile_pool(name="hT", bufs=2))
        bpool = ctx.enter_context(tc.tile_pool(name="bias", bufs=1))
        gpool = ctx.enter_context(tc.tile_pool(name="gath", bufs=2))
        dpool = ctx.enter_context(tc.tile_pool(name="dj", bufs=1))
        jpool = ctx.enter_context(tc.tile_pool(name="jt", bufs=3))
        epool = ctx.enter_context(tc.tile_pool(name="et", bufs=2))
        pspool = ctx.enter_context(tc.tile_pool(name="ps", bufs=2, space="PSUM"))
        acc = ctx.enter_context(tc.tile_pool(name="acc", bufs=1))

        # cols 0:128 = (m, g) accumulators; 128:136 = block-0 first-group
        # second-half partials; 136 = last-tile second-half partial
        s_cols = acc.tile([P, MG * NG + MBB + 1], F32, tag="scols")
        s_fin = acc.tile([P, MG], F32, tag="sfin")
        t_fin = acc.tile([P, MR], F32, tag="tfin")
        tpart = acc.tile([P, MR * 4], F32, tag="tpart")
        DC = D // 4             # row-dot chunk width

        wv = wpool.tile([P, KP, 2, VC], FP8, tag="w")
        bb = bpool.tile([P, VC], F32, tag="bias")

        def load_h(b):
            ht = hpool.tile([P, KP, 2, TB], FP8, tag="hT")
            for kp in range(KP):
                nc.gpsimd.dma_start(
                    ht[:, kp, :, :], h_r[kp][:, :, b * TB:(b + 1) * TB])
            return ht

        # Issue order on the gpsimd queue == DMA order.  The first psum
        # tile needs W group 0 + the first 128 tokens of hidden + bias
        # group 0; everything else overlaps with compute.
        ht_cur = hpool.tile([P, KP, 2, TB], FP8, tag="hT")

        def _dma_w(lo, hi):
            for kp in range(KP):
                nc.gpsimd.dma_start(
                    wv[:, kp, :, lo:hi], W_r[kp][:, :, lo:hi])

        def _dma_h0(lo, hi):
            for kp in range(KP):
                nc.gpsimd.dma_start(
                    ht_cur[:, kp, :, lo:hi], h_r[kp][:, :, lo:hi])

        def _dma_bias(g):
            nc.gpsimd.dma_start(
                bb[:, g * GV:(g + 1) * GV],
                bias_d[g * GV:(g + 1) * GV].partition_broadcast(P))

        # Startup DMA order (gpsimd queue == transfer order).  Chosen by
        # simulator sweep; the first psum tile needs W group 0 + the first
        # 128 tokens of hidden + bias group 0.
        for step in STARTUP_ORDER:
            kind, a = step
            if kind == "w":
                _dma_w(*a)
            elif kind == "h":
                _dma_h0(*a)
            else:
                _dma_bias(a)

        def half_tile_pass(ht, pt, voff, msubs, cols, fuse=False):
            """Two 2-bank sub-tiles (one per (m, vocab-half)) sharing one
            4-bank psum tile: matmul sweeps first, then the drains.  With
            fuse=True (both subs over the SAME vocab half for different
            m-blocks) the bias-add runs as one 4-bank DVE op against a
            stride-0-repeated bias view, keeping the DVE drain rate equal
            to the main loop's."""
            for i, (m, bk0) in enumerate(msubs):
                for kp in range(KP):
                    lhsT = ht[:, kp, :, m * P:(m + 1) * P]
                    for bk in range(2):
                        nc.tensor.matmul(
                            pt[:, 2 * i + bk, 0:BANK], lhsT,
                            wv[:, kp, :,
                               voff + (bk0 + bk) * BANK:
                               voff + (bk0 + bk + 1) * BANK],
                            start=(kp == 0), stop=(kp == KP - 1),
                            perf_mode=mybir.MatmulPerfMode.DoubleRow)
            if fuse:
                bk0 = msubs[0][1]
                bbv4 = bb[:, voff + bk0 * BANK:voff + (bk0 + 2) * BANK] \
                    .rearrange("p (o k c) -> p o k c", o=1, c=BANK) \
                    .broadcast_to([P, 2, 2, BANK])
                jt = jpool.tile([P, BANKS, BANK], BF16, tag="jt")
                nc.vector.tensor_tensor(
                    jt[:].rearrange("p (a k) c -> p a k c", a=2),
                    pt[:, 0:BANKS, 0:BANK].rearrange(
                        "p (a k) c -> p a k c", a=2),
                    bbv4, op=Alu.add)
                et = epool.tile([P, BANKS, BANK], BF16, tag="et")
                for i in range(2):
                    nc.scalar.activation(
                        et[:, 2 * i:2 * i + 2, :], jt[:, 2 * i:2 * i + 2, :],
                        Act.Exp, accum_out=s_cols[:, cols[i]:cols[i] + 1])
                return
            for i, (m, bk0) in enumerate(msubs):
                bb2 = bb[:, voff + bk0 * BANK:voff + (bk0 + 2) * BANK]
                jt = jpool.tile([P, 2, BANK], BF16, tag="jt2")
                nc.vector.tensor_tensor(
                    jt[:], pt[:, 2 * i:2 * i + 2, 0:BANK],
                    bb2.rearrange("p (k c) -> p k c", c=BANK), op=Alu.add)
                et = epool.tile([P, 2, BANK], BF16, tag="et2")
                nc.scalar.activation(
                    et[:], jt[:], Act.Exp,
                    accum_out=s_cols[:, cols[i]:cols[i] + 1])

        # Exact-f32 target-logit row-dot, chopped into D/4-wide chunks that
        # slot into the per-drain DVE slack (PE fills a psum tile in 3.33us,
        # the drain takes 2.2us; each 0.7us chunk fits the gap).  The
        # multiply chunks for block b run during b's g1 drains; the reduce
        # chunks run during block b+1's g0 drains (half-block lag so the
        # hg/wg DMAs always arrive in time).
        dj_prev = None
        for b in range(NB):
            ht = ht_cur
            hg = gpool.tile([P, D], F32, tag="hg")
            nc.gpsimd.dma_start(hg[:], hn_d[b * P:(b + 1) * P, :])
            wgt = gpool.tile([P, D], F32, tag="wg")
            nc.gpsimd.dma_start(wgt[:], wg_d[b * P:(b + 1) * P, :])
            ht_next = load_h(b + 1) if b + 1 < NB else None
            dj = dpool.tile([P, D], F32, tag="dj")

            for g in range(NG):
                if b == 0 and g == 0:
                    # Block 0's first group runs as half-width (1000-col)
                    # sub-tiles so PE can start after only the first 1000
                    # W columns have arrived.  The second half accumulates
                    # into scratch cols 128+m, folded in during block 1.
                    for half in range(2):
                        for mp in range(MBB // 2):
                            pt = pspool.tile([P, BANKS, 512], F32, tag="ps")
                            ms = [(2 * mp, 2 * half), (2 * mp + 1, 2 * half)]
                            cols = [(2 * mp + s) * NG if half == 0
                                    else MG * NG + 2 * mp + s
                                    for s in range(2)]
                            half_tile_pass(ht, pt, 0, ms, cols, fuse=True)
                    continue
                bbv = bb[:, g * GV:(g + 1) * GV].rearrange(
                    "p (k c) -> p k c", c=BANK)
                for m in range(MBB):
                    last = b == NB - 1
                    if last and g == 1 and m == MBB - 1:
                        # last tile: bank-major halves so the closing drain
                        # is half-size (shorter tail chain); second half
                        # accumulates into scratch col 136.
                        pt = pspool.tile([P, BANKS, 512], F32, tag="ps")
                        col = (b * MBB + m) * NG + g
                        half_tile_pass(ht, pt, GV, [(m, 0), (m, 2)],
                                       [col, MG * NG + MBB])
                        continue
                    pt = pspool.tile([P, BANKS, 512], F32, tag="ps")
                    for kp in range(KP):
                        lhsT = ht[:, kp, :, m * P:(m + 1) * P]
                        for bk in range(BANKS):
                            nc.tensor.matmul(
                                pt[:, bk, 0:BANK], lhsT,
                                wv[:, kp, :,
                                   g * GV + bk * BANK:g * GV + (bk + 1) * BANK],
                                start=(kp == 0), stop=(kp == KP - 1),
                                perf_mode=mybir.MatmulPerfMode.DoubleRow)
                    jt = jpool.tile([P, BANKS, BANK], BF16, tag="jt")
                    nc.vector.tensor_tensor(
                        jt[:], pt[:, 0:BANKS, 0:BANK], bbv, op=Alu.add)
                    et = epool.tile([P, BANKS, BANK], BF16, tag="et")
                    col = (b * MBB + m) * NG + g
                    nc.scalar.activation(
                        et[:], jt[:], Act.Exp,
                        accum_out=s_cols[:, col:col + 1])
                    # row-dot chunks in the drain slack.  Normally: muls of
                    # block b in b's g1 slots, reduces in b+1's g0 slots.
                    # The last block pulls both into its own slots so the
                    # tail has no row-dot work left.
                    last = b == NB - 1
                    if g == 0 and m < 4 and dj_prev is not None:
                        c = slice(m * DC, (m + 1) * DC)
                        nc.vector.tensor_reduce(
                            tpart[:, (b - 1) * 4 + m:(b - 1) * 4 + m + 1],
                            dj_prev[:, c], axis=mybir.AxisListType.X,
                            op=Alu.add)
                    if (g == 0 and 4 <= m if last else g == 1 and m < 4):
                        mm = m - 4 if last else m
                        c = slice(mm * DC, (mm + 1) * DC)
                        nc.vector.tensor_mul(dj[:, c], hg[:, c], wgt[:, c])
                    if b == 1 and g == 1 and m == 4:
                        # fold block-0 first-group second-half partials
                        # (scratch cols) into their (m, g0) accumulators
                        sv0 = s_cols[:, 0:MG * NG].rearrange(
                            "p (m g) -> p m g", g=NG)
                        nc.vector.tensor_tensor(
                            sv0[:, 0:MBB, 0], sv0[:, 0:MBB, 0],
                            s_cols[:, MG * NG:MG * NG + MBB], op=Alu.add)
                    if last and g == 1 and m < 4:
                        c = slice(m * DC, (m + 1) * DC)
                        nc.vector.tensor_reduce(
                            tpart[:, b * 4 + m:b * 4 + m + 1],
                            dj[:, c], axis=mybir.AxisListType.X, op=Alu.add)
            dj_prev = dj
            ht_cur = ht_next

        tv = tpart[:].rearrange("p (m c) -> p m c", c=4)
        nc.vector.tensor_reduce(
            t_fin[:], tv, axis=mybir.AxisListType.X, op=Alu.add)
        nc.sync.dma_start(t_out[:], t_fin[:])
        sv = s_cols[:, 0:MG * NG].rearrange("p (m g) -> p m g", g=NG)
        nc.vector.tensor_tensor(s_fin[:], sv[:, :, 0], sv[:, :, 1], op=Alu.add)
        # last-tile second-half partial (scratch col 136)
        nc.vector.tensor_tensor(
            s_fin[:, MG - 1:MG], s_fin[:, MG - 1:MG],
            s_cols[:, MG * NG + MBB:MG * NG + MBB + 1], op=Alu.add)
        nc.sync.dma_start(s_out[:], s_fin[:])

    nc.compile()
    return nc


_NC_CACHE = {}


def _get_program():
    if "v2" not in _NC_CACHE:
        _NC_CACHE["v2"] = _build()
    return _NC_CACHE["v2"]


def kernel(hidden_states, head_weight, head_bias, loss_weight, labels,
           chunk_size=None, **_unused):
    hidden = np.asarray(hidden_states, dtype=np.float32)
    W = np.asarray(head_weight, dtype=np.float32)
    bias = np.asarray(head_bias, dtype=np.float32)
    lw = np.asarray(loss_weight, dtype=np.float32)
    labels = np.asarray(labels).astype(np.int64)

    assert hidden.shape == (N_TOK, D) and W.shape == (V, D)

    nc = _get_program()
    ht = np.ascontiguousarray(hidden.T)            # [D, N]
    Wt = np.ascontiguousarray(W.T)                 # [D, V]
    Wg = W[labels]                                 # gathered rows [N, D]
    in_maps = []
    for c in range(N_CORES):
        vsl = slice(c * VC, (c + 1) * VC)
        tsl = slice(c * TC, (c + 1) * TC)
        in_maps.append(dict(
            h=ht,
            W=np.ascontiguousarray(Wt[:, vsl]),
            bias=np.ascontiguousarray(bias[vsl]),
            hn=np.ascontiguousarray(hidden[tsl]),
            wg=np.ascontiguousarray(Wg[tsl])))
    res = run_bass_kernel_spmd(nc, in_maps, list(range(N_CORES)))

    # unshard + host-side scalar combine (the "all_reduce" of the hint):
    # sum the per-core partial exp-sums over vocab shards, then the
    # weighted-mean reduction over tokens.
    s = np.zeros((P, MG), dtype=np.float64)
    for r in res.results:
        s += r["s_out"].astype(np.float64)
    s = s.T.reshape(-1)                            # token-ordered [N]
    tgt = np.concatenate([r["t_out"].T.reshape(-1) for r in res.results])
    tgt = tgt.astype(np.float64) + bias[labels].astype(np.float64)
    lse = np.log(s)
    nll = lse - tgt
    w64 = lw.astype(np.float64)
    loss = (w64 * nll).sum() / max(w64.sum(), 1.0)
    return np.float32(loss)
